# revision 12
# baseline (speedup 1.0000x reference)
"""Trainium2 Bass kernel for nn_Conditioned_Mlp (moe_routing).

Computation (reference):
    h      = relu(q @ W1[e] + b1[e])          [N, E, H]
    q_pred = h @ W2[e] + b2[e]                [N, E, D]
    gate   = softmax(concat(q, k) @ Wg + bg)  [N, E]
    out    = sum_e gate[:, e] * q_pred[:, :, e]

Sharding: pure data-parallel over N across 8 cores (2048 tokens/core);
all weights replicated.  Per core the kernel processes tokens in tiles
of 512, keeps h in transposed layout [H, tok] so layer-2 consumes it as
the stationary operand directly, and fuses gate/softmax/combine on-chip.
Matmuls run in bf16 (fp32 PSUM accumulation).

Gate path: logits are accumulated in transposed [4, tok] layout with the
tiny Wg chunks as the stationary operand (cheap LDWEIGHTS, full-width
512-token streams), exp'd with the bias folded into the activation, and
the softmax denominator is folded into one per-token DVE scale applied
to the final combined output, so the per-expert paths use the
unnormalized gates directly.

Host-side work: dtype conversion to bf16 and weight-layout reordering so
every DMA the device issues is fully contiguous.
"""

import sys

sys.path.insert(0, "/opt/trn_rl_repo")

from contextlib import ExitStack

import ml_dtypes
import numpy as np

import concourse.bass as bass
import concourse.mybir as mybir
import concourse.tile as tile
from concourse import bacc
from concourse.bass import ds, ts
from concourse.bass_utils import run_bass_kernel_spmd

BF16 = mybir.dt.bfloat16
F32 = mybir.dt.float32
AF = mybir.ActivationFunctionType
ALU = mybir.AluOpType

N, D, E, H = 16384, 1024, 4, 4096
NCORES = 8
NT = N // NCORES  # tokens per core (2048)
TT = 512          # tokens per tile
NTT = NT // TT    # token tiles per core (4)
NT128 = TT // 128 # 128-token chunks per tile (4)
DC = D // 128     # contraction chunks over D (8)
HC = H // 128     # h-chunks (32)
HG = H // 512     # W1 streaming groups over H (8)

_CACHE = {}


def _build(trace_sim=False, repeat=1):
    nc = bacc.Bacc("TRN2", target_bir_lowering=False)

    # qtr[t, p, j, tok] = q[t*TT + tok, j*128 + p]  (host pre-transposed)
    q = nc.dram_tensor("qtr", [NTT, 128, DC, TT], BF16, kind="ExternalInput")
    k = nc.dram_tensor("ktr", [NTT, 128, DC, TT], BF16, kind="ExternalInput")
    # w1r[e, hg, p, d*512+s] = W1[e, d*128+p, hg*512+s]
    w1 = nc.dram_tensor("w1r", [E, HG, 128, DC * 512], BF16, kind="ExternalInput")
    # w2r[e, p, c, d] = W2[e, c*128+p, d]
    w2 = nc.dram_tensor("w2r", [E, 128, HC, D], BF16, kind="ExternalInput")
    # b1r[p, e*HC+c] = b1[e, c*128+p]
    b1 = nc.dram_tensor("b1r", [128, E * HC], F32, kind="ExternalInput")
    b2 = nc.dram_tensor("b2", [1, E * D], BF16, kind="ExternalInput")
    # wgr[p, j, g] = Wg[j*128+p, g]   (j < DC: q part; j >= DC: k part)
    wg = nc.dram_tensor("wgr", [128, 2 * DC, 4], BF16, kind="ExternalInput")
    # bgT[e, 0] = bg[e]
    bg = nc.dram_tensor("bgT", [4, 1], F32, kind="ExternalInput")
    ident = nc.dram_tensor("ident4", [4, 4], BF16, kind="ExternalInput")
    out = nc.dram_tensor("out", [NT, D], F32, kind="ExternalOutput")

    with ExitStack() as ctx:
        tc = ctx.enter_context(tile.TileContext(nc, trace_sim=trace_sim))
        const = ctx.enter_context(tc.tile_pool(name="const", bufs=1))
        qkp = ctx.enter_context(tc.tile_pool(name="qk", bufs=2))
        w1p = ctx.enter_context(tc.tile_pool(name="w1p", bufs=3))
        w2p = ctx.enter_context(tc.tile_pool(name="w2p", bufs=1))
        htp = ctx.enter_context(tc.tile_pool(name="htp", bufs=2))
        yp = ctx.enter_context(tc.tile_pool(name="yp", bufs=1))
        gTp = ctx.enter_context(tc.tile_pool(name="gTp", bufs=2))
        gp = ctx.enter_context(tc.tile_pool(name="gp", bufs=8))
        ps1 = ctx.enter_context(tc.tile_pool(name="ps1", bufs=4, space="PSUM"))
        ps2 = ctx.enter_context(tc.tile_pool(name="ps2", bufs=4, space="PSUM"))

        # consts ride the ACT HWDGE queue so the SP queue's head is free
        # for the first qT chunk + W1 group (the PE-start critical path)
        wg_sb = const.tile([128, 2 * DC, 4], BF16)
        nc.scalar.dma_start(out=wg_sb, in_=wg[:, :, :])
        b1_sb = const.tile([128, E * HC], F32)
        nc.scalar.dma_start(out=b1_sb, in_=b1[:, :])
        b2_sb = const.tile([4, D], BF16)
        nc.scalar.dma_start(out=b2_sb, in_=b2[:, :].rearrange("p (e d) -> (p e) d", e=E))
        bg_sb = const.tile([4, 1], F32)
        nc.scalar.dma_start(out=bg_sb, in_=bg[:, :])
        ident_sb = const.tile([4, 4], BF16)
        nc.scalar.dma_start(out=ident_sb, in_=ident[:, :])

        for _rep in range(repeat):
          # First-tile q: chunked per-d DMAs with the first W1 group
          # interleaved right after chunk 0, so the PE's first layer-1
          # chain starts as soon as ~1.5 MB (not 5 MB) has landed.
          qT0 = qkp.tile([128, DC, TT], BF16, tag="qT")
          kT0 = qkp.tile([128, DC, TT], BF16, tag="kT")
          # Interleave the first three W1 groups between the qT chunks on
          # the SP queue: layer-1's chains need qT chunk d for EVERY hg
          # group, and hg_i is consumed ~6.9 us apart, so this ordering
          # keeps the PE fed through the whole DMA ramp.
          nc.sync.dma_start(out=qT0[:, 0, :], in_=q[0, :, 0, :])
          w1_pre = {
              hg: w1p.tile([128, DC * 512], BF16, tag="w1", name=f"w1pre{hg}")
              for hg in range(3)
          }
          nc.sync.dma_start(out=w1_pre[0][:, :], in_=w1[0, 0, :, :])
          for d in range(1, DC):
              nc.sync.dma_start(out=qT0[:, d, :], in_=q[0, :, d, :])
              if d == 3:
                  nc.sync.dma_start(out=w1_pre[1][:, :], in_=w1[0, 1, :, :])
              elif d == 5:
                  nc.sync.dma_start(out=w1_pre[2][:, :], in_=w1[0, 2, :, :])
          # kT is first needed by the t0 gate block (~55 us in); keep it
          # off the SP queue AND delay it so the critical qT0+W1 stream
          # gets the full HBM bandwidth during the ramp.  Chunked so the
          # gate chain can consume j-chunks as they land.
          with tc.tile_wait_until(0.032):
              for d in range(DC):
                  nc.scalar.dma_start(out=kT0[:, d, :], in_=k[0, :, d, :])
          qk_next = (qT0, kT0)

          for t in range(NTT):
              tok0 = t * TT
              # qT[p, j, tok] = q[tok0+tok, j*128+p]
              qT, kT = qk_next

              y = yp.tile([128, NT128, D], F32, tag="y")
              gates = None   # unnormalized exp(logits) [128tok, 4] per t4
              recips = None  # 1/sum_e exp(logits) [128tok, 1] per t4
              gexpT = None   # unnormalized exp(logits) [4, tok] bf16

              def gate_logits():
                  # logitsT[e, tok] accumulated with Wg chunks stationary:
                  # LDWEIGHTS is 4 columns (cheap), streams are 512 wide.
                  pgT = ps1.tile([4, TT], F32, tag="l1")
                  for j in range(DC):
                      nc.tensor.matmul(
                          pgT, lhsT=wg_sb[:, j, :], rhs=qT[:, j, :],
                          start=(j == 0), stop=False,
                      )
                  for j in range(DC):
                      nc.tensor.matmul(
                          pgT, lhsT=wg_sb[:, DC + j, :], rhs=kT[:, j, :],
                          start=False, stop=(j == DC - 1),
                      )
                  gT = gTp.tile([4, TT], BF16, tag="gexpT")
                  # logits ~N(0,1); exp cannot overflow, skip max-subtraction
                  nc.scalar.activation(gT, pgT, AF.Exp, bias=bg_sb[:, :])
                  return gT

              def gate_finish(gexpT):
                  # Per-t4: transpose exp(logits) to token-partition layout,
                  # row-sum on the evacuation copy, reciprocal for the final
                  # normalization scale.  Also seed y with the gate-weighted
                  # b2 so per-expert layer-2 paths skip their bias entirely.
                  gates, recips = [], []
                  for t4 in range(NT128):
                      pt = ps1.tile([128, 4], BF16, tag="l1")
                      nc.tensor.transpose(pt, gexpT[:, ts(t4, 128)], ident_sb)
                      gexp = gp.tile([128, 4], F32, tag="gexp")
                      gsum = gp.tile([128, 1], F32, tag="gsum")
                      nc.scalar.activation(gexp, pt, AF.Copy, accum_out=gsum)
                      grec = gp.tile([128, 1], F32, tag="grec")
                      nc.vector.reciprocal(grec, gsum)
                      gate = gp.tile([128, 4], F32, tag="gate")
                      nc.vector.tensor_scalar_mul(gate, gexp, grec)
                      gates.append(gate)
                      recips.append(grec)
                  for dh in range(2):
                      for t4 in range(NT128):
                          pb = ps2.tile([128, 512], F32, tag="l2")
                          nc.tensor.matmul(
                              pb, lhsT=gexpT[:, ts(t4, 128)],
                              rhs=b2_sb[:, ds(dh * 512, 512)],
                              start=True, stop=True,
                          )
                          # seed used the unnormalized gates; fold the
                          # softmax denominator in on the evacuation copy
                          nc.scalar.activation(
                              y[:, t4, ds(dh * 512, 512)], pb, AF.Copy,
                              scale=recips[t4][:, :],
                          )
                  return gates, recips

              if t > 0:
                  # qT/kT prefetched long ago: compute the gate logits up
                  # front so the finish block never stalls the PE later.
                  gexpT = gate_logits()

              # ---- experts
              for e in range(E):
                  # W1 groups stream on the SP HWDGE queue; the 8 MB W2 load
                  # goes through the GPSIMD SWDGE queue (the GpSimd engine is
                  # otherwise idle) so it can never head-of-line-block W1.
                  if t == 0 and e == 0:
                      w1t_hg0 = w1_pre[0]
                  else:
                      w1t_hg0 = w1p.tile([128, DC * 512], BF16, tag="w1")
                      nc.sync.dma_start(out=w1t_hg0[:, :], in_=w1[e, 0, :, :])
                  w2t = w2p.tile([128, HC, D], BF16, tag="w2")
                  if t == 0 and e == 0:
                      # first tile: delay + split W2 by d-halves so the 8 MB
                      # can't starve the qT0/kT0/W1 ramp (layer-2 consumes
                      # the dh=0 half ~25 us before the dh=1 half)
                      for dh, wait in ((0, 0.018), (1, 0.038)):
                          with tc.tile_wait_until(wait):
                              for j in range(8):
                                  nc.gpsimd.dma_start(
                                      out=w2t[:, ds(j * 4, 4), ds(dh * 512, 512)],
                                      in_=w2[e, :, ds(j * 4, 4), ds(dh * 512, 512)],
                                  )
                  else:
                      for j in range(8):
                          nc.gpsimd.dma_start(
                              out=w2t[:, ds(j * 4, 4), :], in_=w2[e, :, ds(j * 4, 4), :]
                          )
                  if e == 2 and t + 1 < NTT:
                      # prefetch next token tile's transposed q/k mid-expert,
                      # away from the weight-critical boundary windows
                      qTn = qkp.tile([128, DC, TT], BF16, tag="qT")
                      kTn = qkp.tile([128, DC, TT], BF16, tag="kT")
                      nc.scalar.dma_start(out=qTn[:, :, :], in_=q[t + 1, :, :, :])
                      nc.scalar.dma_start(out=kTn[:, :, :], in_=k[t + 1, :, :, :])
                      qk_next = (qTn, kTn)
                  ht = htp.tile([128, HC, TT], BF16, tag="ht")
                  # layer 1: hT[p, c, tok] = relu(q @ W1 + b1)[tok, c*128+p]
                  for hg in range(HG):
                      if hg == 0:
                          w1t = w1t_hg0
                      elif t == 0 and e == 0 and hg in w1_pre:
                          w1t = w1_pre[hg]
                      else:
                          w1t = w1p.tile([128, DC * 512], BF16, tag="w1")
                          nc.sync.dma_start(out=w1t[:, :], in_=w1[e, hg, :, :])
                      for hs in range(4):
                          hc = hg * 4 + hs
                          p1 = ps1.tile([128, TT], F32, tag="l1")
                          for d in range(DC):
                              nc.tensor.matmul(
                                  p1,
                                  lhsT=w1t[:, ds(d * 512 + hs * 128, 128)],
                                  rhs=qT[:, d, :],
                                  start=(d == 0),
                                  stop=(d == DC - 1),
                              )
                          nc.scalar.activation(
                              ht[:, hc, :], p1, AF.Relu,
                              bias=b1_sb[:, e * HC + hc : e * HC + hc + 1],
                          )
                      if e == 0 and hg == 0 and t > 0:
                          # exp(logitsT) is ready by now; finish the gate
                          # while layer-1 streams (zero PE head-of-line wait)
                          gates, recips = gate_finish(gexpT)
                  if e == 0 and t == 0:
                      # first tile: kT lands too late to lead with the gate;
                      # compute it after e0's layer 1 as before
                      gexpT = gate_logits()
                      gates, recips = gate_finish(gexpT)
                  # layer 2 + gated accumulation into y.  t4-outer keeps all
                  # 32 accumulation matmuls on ONE psum bank back-to-back —
                  # per-instruction bank cycling triggers the documented HAM
                  # micro-idle oscillation (~45% PE throughput loss).
                  for dh in range(2):
                      for t4 in range(NT128):
                          p2 = ps2.tile([128, 512], F32, tag="l2")
                          for h in range(HC):
                              nc.tensor.matmul(
                                  p2,
                                  lhsT=ht[:, h, ts(t4, 128)],
                                  rhs=w2t[:, h, ds(dh * 512, 512)],
                                  start=(h == 0),
                                  stop=(h == HC - 1),
                              )
                          g_col = gates[t4][:, e : e + 1]
                          ysl = y[:, t4, ds(dh * 512, 512)]
                          nc.vector.scalar_tensor_tensor(
                              out=ysl, in0=p2, scalar=g_col, in1=ysl,
                              op0=ALU.mult, op1=ALU.add,
                          )
                          if e == E - 1:
                              # stream out per (dh, t4) as soon as it lands
                              nc.scalar.dma_start(
                                  out=out[
                                      tok0 + t4 * 128 : tok0 + (t4 + 1) * 128,
                                      ds(dh * 512, 512),
                                  ],
                                  in_=ysl,
                              )

    nc.compile()
    return nc


def _get_nc():
    if "nc" not in _CACHE:
        _CACHE["nc"] = _build()
    return _CACHE["nc"]


def _prep_inputs(q, k, W1, b1, W2, b2, Wg, bg):
    bf16 = ml_dtypes.bfloat16
    q = np.asarray(q, dtype=np.float32)
    k = np.asarray(k, dtype=np.float32)
    W1 = np.asarray(W1, dtype=np.float32)
    b1 = np.asarray(b1, dtype=np.float32)
    W2 = np.asarray(W2, dtype=np.float32)
    b2 = np.asarray(b2, dtype=np.float32)
    Wg = np.asarray(Wg, dtype=np.float32)
    bg = np.asarray(bg, dtype=np.float32)

    # per-core pre-transposed q/k: [NTT, 128, DC, TT]
    def tr(x):
        xc = x.astype(bf16).reshape(NCORES, NTT, TT, DC, 128)
        return np.ascontiguousarray(xc.transpose(0, 1, 4, 3, 2))

    qtr = tr(q)
    ktr = tr(k)
    w1r = np.ascontiguousarray(
        W1.astype(bf16).reshape(E, DC, 128, HG, 512).transpose(0, 3, 2, 1, 4)
    ).reshape(E, HG, 128, DC * 512)
    w2r = np.ascontiguousarray(
        W2.astype(bf16).reshape(E, HC, 128, D).transpose(0, 2, 1, 3)
    )
    b1r = np.ascontiguousarray(
        b1.reshape(E, HC, 128).transpose(2, 0, 1).reshape(128, E * HC)
    )
    wgr = np.ascontiguousarray(
        Wg.astype(bf16).reshape(2 * DC, 128, 4).transpose(1, 0, 2)
    )
    bgT = np.ascontiguousarray(bg.astype(np.float32).reshape(4, 1))

    in_maps = []
    for c in range(NCORES):
        in_maps.append(
            {
                "qtr": qtr[c],
                "ktr": ktr[c],
                "w1r": w1r,
                "w2r": w2r,
                "b1r": b1r,
                "b2": np.ascontiguousarray(b2.astype(bf16).reshape(1, E * D)),
                "wgr": wgr,
                "bgT": bgT,
                "ident4": np.eye(4, dtype=bf16),
            }
        )
    return in_maps


def run(inputs, trace=False):
    """Run the kernel; returns (output, BassKernelResults)."""
    in_maps = _prep_inputs(**inputs)
    res = run_bass_kernel_spmd(
        _get_nc(), in_maps, core_ids=list(range(NCORES)), trace=trace
    )
    out = np.concatenate([r["out"] for r in res.results], axis=0)
    return out, res


def kernel(**inputs):
    out, _ = run(inputs, trace=False)
    return out
